# revision 74
# baseline (speedup 1.0000x reference)
"""2-layer GCN (PyG GCNConv x2 + relu + log_softmax) on 8 Trainium2 NeuronCores.

Strategy: shard destination nodes (and their incoming edges) across the 8
cores. Each layer:
  1. dense  h = x @ W  data-parallel over the core's node shard,
     scaled to g = dinv * h  (dinv = 1/sqrt(weighted in-degree + self loop))
  2. AllGather the g-shards into a replicated bf16 [N, C] table
  3. edge pass: dma_gather g[src] rows (256B bf16) for the core's
     (dst-sorted, padded) edges, build bf16 selection matrices
     S^T[e, n] = w_e * (dst_local[e] == n) on the vector engine,
     segment-sum via bf16 TensorE matmuls accumulated in PSUM per
     128-node destination tile
  4. epilogue out[n] = dinv[n] * (psum[n] + g[n]) + b  (+relu / log_softmax)

dma_gather uses int16 indices and the serialized DMA engines are the
bottleneck resource, so the node table is split in THREE regions by source
position (A: tiles 0-24, B1: 25-36, B2: 37-48).  Each dst-tile's edge list
is partitioned per source region and padded to a multiple of 128 (uniform
group counts across cores/tiles so one SPMD program serves all cores; a
host-side balancer keeps the per-tile counts under (9,4,4) groups).
Region tables AllGather progressively (after dense/epilogue tile 24/36/48),
which lets layer 2's region passes start while layer 1 is still streaming:
layer 2 is split into three partial passes (A, B1, B2; partials staged in
o_sb) interleaved into layer 1's tile loop, so the DMA queue never drains
until the final B2 tail.  Tables and outputs use a partition-major row
order (row = p*tiles + t) so every bulk DMA moves contiguous per-partition
spans.  Layer 2's 64-channel rows are zero-padded to 128 bf16 channels to
satisfy the 256B-per-descriptor gather minimum.
"""
import sys

sys.path.insert(0, "/opt/trn_rl_repo")

import numpy as np
import ml_dtypes

from concourse import bass, mybir, bacc
import concourse.tile as tile
from concourse.bass_utils import run_bass_kernel_spmd

N = 50000
E = 800000
INCH = 128
HID = 128
OUT = 64
NCORES = 8
NSH = N // NCORES          # 6250 nodes per shard
P = 128
NT = (NSH + P - 1) // P    # 49 dst tiles per shard
NPAD = NT * P              # 6272

# source-position regions (by tile range); region tables must stay int16
REGT = ((0, 25), (25, 37), (37, 49))        # tile ranges [t0, t1)
RROWS = tuple((t1 - t0) * P for t0, t1 in REGT)   # 3200, 1536, 1536
RBASE = (0, 3200, 4736)                     # position offsets
NREG = 3
KCAP = (9, 4, 4)                            # balancer targets (groups/tile)
CHS = (24, 16, 16)                          # gather chunk size per region
LAGS = (27, 40, 51)                         # layer-2 pass trail (tiles)

f32 = mybir.dt.float32
bf16 = mybir.dt.bfloat16
i16 = mybir.dt.int16
i32 = mybir.dt.int32
AF = mybir.ActivationFunctionType
ALU = mybir.AluOpType

npbf16 = ml_dtypes.bfloat16

_PROGRAM_CACHE = {}


def _build_program(KT, trace=False, collectives=True, skip=(), with_bias=True):
    """KT: per-region tuple of per-tile group counts."""
    key = (tuple(map(tuple, KT)), collectives, tuple(skip), with_bias)
    if key in _PROGRAM_CACHE:
        return _PROGRAM_CACHE[key]

    GOFF = []                 # group offset of (r, t); region-major layout
    base = 0
    RG0 = []                  # first group of each region
    for r in range(NREG):
        RG0.append(base)
        offs = []
        for tt in range(NT):
            offs.append(base)
            base += KT[r][tt]
        GOFF.append(offs)
    RGN = [sum(KT[r]) for r in range(NREG)]   # groups per region
    TP = base
    EPAD = TP * P

    nc = bacc.Bacc("TRN2", target_bir_lowering=False, debug=False,
                   enable_asserts=True, num_devices=NCORES)

    # inputs
    xT_d = nc.dram_tensor("xT", [P, NPAD], bf16, kind="ExternalInput")
    idxw_d = nc.dram_tensor("idxw", [P, EPAD // 16], i16, kind="ExternalInput")
    dstl_d = nc.dram_tensor("dstlT", [P, TP], f32, kind="ExternalInput")
    wt_d = nc.dram_tensor("wT", [P, TP], f32, kind="ExternalInput")
    dinv_d = nc.dram_tensor("dinv", [P, NT], f32, kind="ExternalInput")
    w1_d = nc.dram_tensor("W1", [INCH, HID], bf16, kind="ExternalInput")
    w2_d = nc.dram_tensor("W2", [HID, OUT], bf16, kind="ExternalInput")
    b1_d = nc.dram_tensor("b1b", [P, HID], f32, kind="ExternalInput")
    b2_d = nc.dram_tensor("b2b", [P, OUT], f32, kind="ExternalInput")
    id_d = nc.dram_tensor("ident", [P, P], bf16, kind="ExternalInput")

    out_d = nc.dram_tensor("out", [P, NT * OUT], f32, kind="ExternalOutput")

    # internal DRAM; shard contributions are partition-major:
    # row = p*(t1-t0) + (t-t0)
    g1_d = [nc.dram_tensor(f"g1r{r}", [RROWS[r], HID], bf16)
            for r in range(NREG)]
    g2_d = [nc.dram_tensor(f"g2r{r}", [RROWS[r], P], bf16)
            for r in range(NREG)]
    t1_d = [nc.dram_tensor(f"t1r{r}", [NCORES * RROWS[r], HID], bf16,
                           addr_space="Shared") for r in range(NREG)]
    t2_d = [nc.dram_tensor(f"t2r{r}", [NCORES * RROWS[r], P], bf16,
                           addr_space="Shared") for r in range(NREG)]

    groups = [list(range(NCORES))]

    with tile.TileContext(nc) as tc:
        with (
            tc.tile_pool(name="pers", bufs=1) as pers,
            tc.tile_pool(name="pst", bufs=2, space="PSUM") as pst,
            tc.tile_pool(name="psd2", bufs=1, space="PSUM") as psd2,
            tc.tile_pool(name="pse1", bufs=2, space="PSUM") as pse1,
            tc.tile_pool(name="pse2", bufs=3, space="PSUM") as pse2,
            tc.tile_pool(name="g1r0", bufs=4) as g1r0,
            tc.tile_pool(name="g1r1", bufs=3) as g1r1,
            tc.tile_pool(name="g1r2", bufs=3) as g1r2,
            tc.tile_pool(name="g2r0", bufs=5) as g2r0,
            tc.tile_pool(name="g2r1", bufs=4) as g2r1,
            tc.tile_pool(name="g2r2", bufs=4) as g2r2,
            tc.tile_pool(name="st1", bufs=2) as st1,
            tc.tile_pool(name="st2", bufs=2) as st2,
            tc.tile_pool(name="ep1", bufs=3) as ep1,
            tc.tile_pool(name="ep2", bufs=3) as ep2,
        ):
            gpools = {(1, 0): g1r0, (1, 1): g1r1, (1, 2): g1r2,
                      (2, 0): g2r0, (2, 1): g2r1, (2, 2): g2r2}

            # persistent tiles: tiny weights first (they gate the dense
            # matmuls), then x (split so the dense phase starts as soon as
            # the low half lands)
            w1 = pers.tile([INCH, HID], bf16)
            nc.sync.dma_start(w1[:], w1_d[:])
            w2 = pers.tile([HID, OUT], bf16)
            nc.sync.dma_start(w2[:], w2_d[:])
            dinv = pers.tile([P, NT], f32)
            nc.sync.dma_start(dinv[:], dinv_d[:])
            xT = pers.tile([P, NPAD], bf16, tag="xT")
            nc.sync.dma_start(xT[:, :RBASE[1]], xT_d[:, :RBASE[1]])
            nc.sync.dma_start(xT[:, RBASE[1]:], xT_d[:, RBASE[1]:])
            # region-A gather indices next (first edge-pass dependency);
            # the remaining metadata is requested after the dense loop so
            # it queues behind the first table write, not ahead of it
            ACOLS = RGN[0] * (P // 16)
            idxw = pers.tile([P, EPAD // 16], i16)
            nc.sync.dma_start(idxw[:, :ACOLS], idxw_d[:, :ACOLS])
            dstlT = pers.tile([P, TP], f32)
            wT = pers.tile([P, TP], f32)
            ident = pers.tile([P, P], bf16)
            if with_bias:
                b1b = pers.tile([P, HID], f32)
                nc.scalar.dma_start(b1b[:], b1_d[:])
                b2b = pers.tile([P, OUT], f32)
                nc.scalar.dma_start(b2b[:], b2_d[:])

            iota_i = pers.tile([P, P], i32, tag="iota_i")
            nc.gpsimd.iota(iota_i[:], pattern=[[1, P]], base=0,
                           channel_multiplier=0)
            cols_b = pers.tile([P, P], bf16)
            nc.vector.tensor_copy(cols_b[:], iota_i[:])

            g1_sb = pers.tile([P, NT * HID], bf16)
            # g2 staged zero-padded to 128 channels (real 64 in low half of
            # each 128-col block); zeroed on the otherwise-idle Pool engine
            g2_sb = pers.tile([P, NT * P], bf16)
            nc.gpsimd.memset(g2_sb[:], 0.0)
            o_sb = pers.tile([P, NT * OUT], bf16)    # layer-2 partials
            oF_sb = pers.tile([P, NT * OUT], f32)    # final log_softmax buf
            negm_sb = pers.tile([P, NT], f32)
            se_sb = pers.tile([P, NT], f32)
            lse_sb = pers.tile([P, NT], f32)

            def share_region(layer, r):
                """AllGather (or local stand-in) region r of layer's table."""
                gsrc = (g1_sb if layer == 1 else g2_sb)
                C = HID if layer == 1 else P
                gd = (g1_d if layer == 1 else g2_d)[r]
                td = (t1_d if layer == 1 else t2_d)[r]
                t0, t1 = REGT[r]
                nc.sync.dma_start(
                    gd[:].rearrange("(p t) c -> p t c", p=P),
                    gsrc[:, t0 * C:t1 * C].rearrange("p (t c) -> p t c", c=C))
                if collectives:
                    nc.gpsimd.collective_compute(
                        "AllGather", ALU.bypass, replica_groups=groups,
                        ins=[gd[:]], outs=[td[:]])
                else:
                    nc.sync.dma_start(td[:RROWS[r], :], gd[:])

            # ---------- dense layer 1: g1 = (dinv*x) @ W1 ----------
            # dinv is folded into x on the host, so this is matmul + copy.
            # Four tiles share one 2KB-wide PSUM bank and drain with a single
            # wide copy (shares pse1's banks: the dense phase is over before
            # the edge pass needs them).
            bounds = sorted(REGT[r][1] for r in range(NREG))
            t = 0
            while t < NT:
                nb = min(b for b in bounds if b > t)
                bt = min(4, nb - t)
                ps = pse1.tile([P, 4 * HID], f32, tag="pse1")
                for i in range(bt):
                    nc.tensor.matmul(ps[:, i * HID:(i + 1) * HID],
                                     lhsT=xT[:, (t + i) * P:(t + i + 1) * P],
                                     rhs=w1[:], start=True, stop=True)
                nc.vector.tensor_copy(
                    g1_sb[:, t * HID:(t + bt) * HID], ps[:, :bt * HID])
                nt = t + bt
                for r in range(NREG):
                    if t < REGT[r][1] <= nt:
                        share_region(1, r)
                t = nt
            nc.sync.dma_start(idxw[:, ACOLS:], idxw_d[:, ACOLS:])
            nc.sync.dma_start(dstlT[:], dstl_d[:])
            nc.sync.dma_start(wT[:], wt_d[:])
            nc.sync.dma_start(ident[:], id_d[:])

            # ---------- edge-pass building blocks ----------
            chunk_cache = {}

            def chunk(layer, r, c):
                """Fetch gather chunk c of (layer, region); cached."""
                keyc = (layer, r, c)
                if keyc in chunk_cache:
                    return chunk_cache[keyc]
                CHr = CHS[r]
                ct = min(CHr, RGN[r] - c * CHr)  # groups in chunk
                buf = gpools[(layer, r)].tile([P, CHr * P], bf16,
                                              tag=f"g{layer}{r}")
                if "gather" not in skip:
                    tab = (t1_d if layer == 1 else t2_d)[r]
                    col0 = (RG0[r] + c * CHr) * (P // 16)
                    nc.gpsimd.dma_gather(
                        out_ap=buf[:, :ct * P].rearrange(
                            "p (k c) -> p k c", c=P),
                        in_ap=tab[:],
                        idxs_ap=idxw[:, col0:col0 + ct * (P // 16)],
                        num_idxs=ct * P,
                        num_idxs_reg=ct * P,
                        elem_size=P,
                        single_packet=False,
                    )
                chunk_cache[keyc] = buf
                return buf

            KM = [max(KT[r]) for r in range(NREG)]

            def build_S(stp, t, r, tag):
                """Selection matrices for tile t's groups of one region."""
                K = KT[r][t]
                st = stp.tile([P, KM[r] * P], bf16, tag=tag, name=f"st{tag}")
                for k in range(K):
                    j = GOFF[r][t] + k
                    if "st" in skip:
                        break
                    nc.vector.tensor_scalar(
                        out=st[:, k * P:(k + 1) * P], in0=cols_b[:],
                        scalar1=dstlT[:, j:j + 1],
                        scalar2=wT[:, j:j + 1],
                        op0=ALU.is_equal, op1=ALU.mult)
                return st

            def mm_region(ps, st, layer, r, t, RW, first, last):
                K = KT[r][t]
                CHr = CHS[r]
                for k in range(K):
                    j = GOFF[r][t] - RG0[r] + k
                    if "mm" in skip:
                        continue
                    buf = chunk(layer, r, j // CHr)
                    slot = j % CHr
                    nc.tensor.matmul(
                        ps, lhsT=st[:, k * P:(k + 1) * P],
                        rhs=buf[:, slot * P:slot * P + RW],
                        start=(first and k == 0),
                        stop=(last and k == K - 1))

            # ---------- per-tile bodies ----------
            def l1slot(t):
                return pse1.tile([P, HID], f32, tag="pse1", name="l1ps")[:]

            def l2slot():
                return pse2.tile([P, OUT], f32, tag="pse2", name="l2ps")[:]

            def l1_tile(t):
                ps = l1slot(t)
                live = [r for r in range(NREG) if KT[r][t] > 0]
                sts = {r: build_S(st1, t, r, f"s1r{r}") for r in live}
                # self-loop term injected into the PSUM chain via TensorE
                nc.tensor.matmul(ps, lhsT=ident[:],
                                 rhs=g1_sb[:, t * HID:(t + 1) * HID],
                                 start=True, stop=(not live))
                for r in live:
                    mm_region(ps, sts[r], 1, r, t, HID,
                              first=False, last=(r == live[-1]))
                # epilogue: relu1 = Relu(dinv * z) in one ACT op; drains ride
                # the Activation engine so DVE only builds S matrices
                relu = ep1.tile([P, HID], bf16, tag="relu1")
                if with_bias:
                    tmp = ep1.tile([P, HID], f32, tag="tmp1")
                    nc.vector.tensor_scalar(
                        out=tmp[:], in0=ps, scalar1=dinv[:, t:t + 1],
                        scalar2=None, op0=ALU.mult)
                    nc.vector.tensor_tensor(
                        out=tmp[:], in0=tmp[:], in1=b1b[:], op=ALU.add)
                    nc.scalar.activation(relu[:], tmp[:], AF.Relu)
                else:
                    nc.scalar.activation(relu[:], ps, AF.Relu,
                                         scale=dinv[:, t:t + 1])
                pt = pst.tile([P, P], bf16, tag="pst")
                nc.tensor.transpose(pt[:], relu[:], ident[:])
                rt = ep1.tile([P, P], bf16, tag="rt")
                nc.scalar.activation(rt[:], pt[:], AF.Identity)

                # fused dense layer 2: g2 = dinv * (relu1 @ W2), zero-padded
                ps2 = psd2.tile([P, OUT], f32, tag="psd2", name="ps2")[:]
                nc.tensor.matmul(ps2, lhsT=rt[:],
                                 rhs=w2[:], start=True, stop=True)
                nc.scalar.activation(g2_sb[:, t * P:t * P + OUT], ps2,
                                     AF.Identity, scale=dinv[:, t:t + 1])
                for r in range(NREG):
                    if t == REGT[r][1] - 1:
                        share_region(2, r)

            def l2_tile(t, r):
                """Layer-2 partial pass for region r; region 2 finalizes.

                Running partials (and the self-loop term) are injected into
                the PSUM chain via identity matmuls; psum drains ride ACT.
                """
                ps = l2slot()
                live = KT[r][t] > 0
                st = build_S(st2, t, r, f"s2r{r}") if live else None
                prev = (g2_sb[:, t * P:t * P + OUT] if r == 0 else
                        o_sb[:, t * OUT:(t + 1) * OUT])
                nc.tensor.matmul(ps, lhsT=ident[:], rhs=prev,
                                 start=True, stop=(not live))
                if live:
                    mm_region(ps, st, 2, r, t, OUT, first=False, last=True)
                if r < 2:
                    nc.scalar.activation(o_sb[:, t * OUT:(t + 1) * OUT], ps,
                                         AF.Identity)
                    return
                # region 2: z complete in PSUM; osl = dinv*z
                osl = oF_sb[:, t * OUT:(t + 1) * OUT]
                nc.vector.tensor_reduce(out=negm_sb[:, t:t + 1], in_=ps,
                                        axis=mybir.AxisListType.X, op=ALU.max,
                                        negate=True)
                nc.vector.tensor_scalar(
                    out=negm_sb[:, t:t + 1], in0=negm_sb[:, t:t + 1],
                    scalar1=dinv[:, t:t + 1], scalar2=None, op0=ALU.mult)
                nc.scalar.activation(osl, ps, AF.Identity,
                                     scale=dinv[:, t:t + 1])
                if with_bias:
                    nc.vector.tensor_tensor(out=osl, in0=osl, in1=b2b[:],
                                            op=ALU.add)
                    nc.vector.tensor_reduce(out=negm_sb[:, t:t + 1], in_=osl,
                                            axis=mybir.AxisListType.X,
                                            op=ALU.max, negate=True)
                    ex = ep2.tile([P, OUT], f32, tag="ex")
                    nc.scalar.activation(ex[:], osl, AF.Exp,
                                         bias=negm_sb[:, t:t + 1],
                                         accum_out=se_sb[:, t:t + 1])
                else:
                    ex = ep2.tile([P, OUT], f32, tag="ex")
                    nc.scalar.activation(ex[:], ps, AF.Exp,
                                         scale=dinv[:, t:t + 1],
                                         bias=negm_sb[:, t:t + 1],
                                         accum_out=se_sb[:, t:t + 1])
                # finalize + write out in chunks so earlier tiles'
                # log_softmax and DMA overlap the rest of the pass
                if t in (24, 36, NT - 1):
                    lo = {24: 0, 36: 25, NT - 1: 37}[t]
                    hi = t + 1
                    nc.scalar.activation(lse_sb[:, lo:hi],
                                         se_sb[:, lo:hi], AF.Ln)
                    for u in range(lo, hi):
                        nc.vector.tensor_scalar(
                            out=oF_sb[:, u * OUT:(u + 1) * OUT],
                            in0=oF_sb[:, u * OUT:(u + 1) * OUT],
                            scalar1=negm_sb[:, u:u + 1],
                            scalar2=lse_sb[:, u:u + 1],
                            op0=ALU.add, op1=ALU.subtract)
                    nc.sync.dma_start(out_d[:, lo * OUT:hi * OUT],
                                      oF_sb[:, lo * OUT:hi * OUT])

            # ---------- interleaved edge passes ----------
            # L1 tiles 0..48; L2's region pass r trails by LAGS[r] tiles
            # (region tables land after L1 tiles 24/36/48 + AllGather).
            for i in range(NT + LAGS[2]):
                if i < NT:
                    l1_tile(i)
                for r in range(NREG):
                    j = i - LAGS[r]
                    if 0 <= j < NT:
                        l2_tile(j, r)

    nc.compile()
    _PROGRAM_CACHE[key] = nc
    return nc


def _wrap_idx(lidx):
    """[EPAD] int -> [128, EPAD//16] int16 (16-partition wrap, 8x replicated)."""
    n = lidx.shape[0]
    w16 = lidx.reshape(n // 16, 16).T.astype(np.int16)   # [16, n/16]
    return np.ascontiguousarray(np.tile(w16, (8, 1)))


def _balance(cnts):
    """Assign a shard's nodes to tiles (each node stays in its own region's
    tile range).  Region-A (dim 0) in-edges are PACKED: early tiles of each
    range are filled to ~KCAP[0]*128 A-edges so later tiles need fewer (or
    zero) A-groups — every core then shares a common descending per-tile
    group profile, which shrinks the padded gather-slot count.  B-region
    counts are capped at KCAP[1/2]*128 per tile.  cnts: [3, NSH] per-node
    incoming-edge counts by source region.  Returns perm."""
    capsv = np.asarray([k * P for k in KCAP], np.int64)
    buckets_all = [[] for _ in range(NT)]
    for r in range(NREG):
        t0, t1 = REGT[r]
        lo = RBASE[r]
        hi = RBASE[r + 1] if r + 1 < NREG else NSH
        ntile = t1 - t0
        cap = np.full(ntile, P, np.int64)
        if t1 == NT:
            cap[ntile - 1] = hi - lo - (ntile - 1) * P
        loads = np.zeros((NREG, ntile), np.int64)
        buckets = [[] for _ in range(ntile)]
        # pack pass: fill tiles in order with the highest-A nodes that fit
        remaining = sorted(range(lo, hi), key=lambda nd: -cnts[0, nd])
        for ti in range(ntile):
            keep = []
            for nd in remaining:
                v = cnts[:, nd]
                if (cap[ti] > 0
                        and loads[0, ti] + v[0] <= capsv[0]
                        and loads[1, ti] + v[1] <= capsv[1]
                        and loads[2, ti] + v[2] <= capsv[2]):
                    buckets[ti].append(nd)
                    cap[ti] -= 1
                    loads[:, ti] += v
                else:
                    keep.append(nd)
            remaining = keep
            # force-fill if the caps blocked everything: among the
            # smallest-A leftovers pick ones that least overflow the B caps
            while cap[ti] > 0 and remaining:
                tail = remaining[-192:]
                tc = cnts[:, tail]
                exc = (np.maximum(loads[1, ti] + tc[1] - capsv[1], 0)
                       + np.maximum(loads[2, ti] + tc[2] - capsv[2], 0)
                       + 0.01 * tc[0])
                pick = len(remaining) - len(tail) + int(np.argmin(exc))
                nd = remaining.pop(pick)
                buckets[ti].append(nd)
                cap[ti] -= 1
                loads[:, ti] += cnts[:, nd]
        # repair: swap nodes between tiles until caps hold on every region
        for _ in range(1500):
            over = np.maximum(loads - capsv[:, None], 0)
            if over.sum() == 0:
                break
            d, u = np.unravel_index(np.argmax(over), over.shape)
            bu = buckets[u]
            bu_s = sorted(bu, key=lambda nd: -cnts[d, nd])[:16]
            done = False
            for v2 in np.argsort(loads[d])[:12]:
                v2 = int(v2)
                if v2 == u:
                    continue
                bv = buckets[v2]
                bv_s = sorted(bv, key=lambda nd: cnts[d, nd])[:16]
                best = None
                oo = (np.maximum(loads[:, u] - capsv, 0).sum()
                      + np.maximum(loads[:, v2] - capsv, 0).sum())
                for n1 in bu_s:
                    for n2 in bv_s:
                        dv = cnts[:, n1] - cnts[:, n2]
                        if dv[d] <= 0:
                            continue
                        no = (np.maximum(loads[:, u] - dv - capsv, 0).sum()
                              + np.maximum(loads[:, v2] + dv - capsv, 0).sum())
                        if no < oo:
                            gain = oo - no
                            if best is None or gain > best[0]:
                                best = (gain, n1, n2)
                if best is not None:
                    _, n1, n2 = best
                    dv = cnts[:, n1] - cnts[:, n2]
                    loads[:, u] -= dv
                    loads[:, v2] += dv
                    bu[bu.index(n1)] = n2
                    bv[bv.index(n2)] = n1
                    done = True
                    break
            if not done:
                break
        for ti in range(ntile):
            buckets_all[t0 + ti] = buckets[ti]
    return np.concatenate([np.asarray(b, np.int64) for b in buckets_all])


def _prep_inputs(x, edge_index, edge_weight):
    src = np.asarray(edge_index[0], dtype=np.int64)
    dst = np.asarray(edge_index[1], dtype=np.int64)
    w = np.asarray(edge_weight, dtype=np.float32)
    x = np.asarray(x, dtype=np.float32)

    deg = np.bincount(dst, weights=w.astype(np.float64), minlength=N)
    deg = deg.astype(np.float32) + 1.0
    dinv = (1.0 / np.sqrt(deg)).astype(np.float32)

    shard_src = src // NSH
    shard_dst = dst // NSH
    # region = src's ORIGINAL local id bucket; the balancer keeps nodes in
    # their region's tile range, so this is permutation-independent.
    lid = src % NSH
    rege = np.where(lid < RBASE[1], 0,
                    np.where(lid < RBASE[2], 1, 2)).astype(np.int64)

    perms = []
    iperms = np.empty((NCORES, NSH), np.int64)
    for s in range(NCORES):
        m = shard_dst == s
        dl = dst[m] - s * NSH
        re = rege[m]
        cnts = np.stack([np.bincount(dl[re == r], minlength=NSH)
                         for r in range(NREG)]).astype(np.int64)
        perm = _balance(cnts)
        perms.append(perm)
        iperms[s][perm] = np.arange(NSH)
    _prep_inputs.last_perms = perms

    per_core = []
    for s in range(NCORES):
        m = shard_dst == s
        es = src[m]
        ew = w[m]
        re = rege[m]
        edp = iperms[s][dst[m] - s * NSH]      # permuted local dst position
        t = edp >> 7
        pos = iperms[shard_src[m], es % NSH]   # permuted position of src
        pt = pos >> 7
        pp = pos & 127
        # table row within its region table, partition-major:
        # row = p*(t1-t0) + (pt - t0)
        t0s = np.asarray([REGT[r][0] for r in range(NREG)])
        wids = np.asarray([REGT[r][1] - REGT[r][0] for r in range(NREG)])
        rows = np.asarray(RROWS)
        trow = (shard_src[m] * rows[re] + pp * wids[re] + (pt - t0s[re]))
        key = (re * NT + t)
        order = np.argsort(key, kind="stable")
        per_core.append((trow[order], edp[order], ew[order],
                         t[order], re[order]))

    # per-tile group counts (max over cores -> one SPMD profile)
    cnt = np.zeros((NREG, NCORES, NT), np.int64)
    for s in range(NCORES):
        es, ed, ew, t, re = per_core[s]
        for r in range(NREG):
            cnt[r, s] = np.bincount(t[re == r], minlength=NT)
    KT = tuple(tuple(int(k) for k in
                     np.ceil(cnt[r].max(axis=0) / P).astype(np.int64))
               for r in range(NREG))
    goff = []          # group offset of (r, t)
    base = 0
    for r in range(NREG):
        offs = []
        for tt in range(NT):
            offs.append(base)
            base += KT[r][tt]
        goff.append(offs)
    TG = base          # total groups

    in_maps = []
    for s in range(NCORES):
        es, ed, ew, t, re = per_core[s]
        lidx_all = np.zeros((TG, P), np.int64)
        dstl_all = np.zeros((TG, P), np.float32)
        w_all = np.zeros((TG, P), np.float32)
        for r in range(NREG):
            hm = re == r
            eh, edh, ewh, th = es[hm], ed[hm], ew[hm], t[hm]
            for tt in range(NT):
                K = KT[r][tt]
                if K == 0:
                    continue
                tm = th == tt
                cc = int(tm.sum())
                row = goff[r][tt]
                flat_l = np.zeros(K * P, np.int64)
                flat_d = np.zeros(K * P, np.float32)
                flat_w = np.zeros(K * P, np.float32)
                flat_l[:cc] = eh[tm]
                flat_d[:cc] = (edh[tm] & 127).astype(np.float32)
                flat_w[:cc] = ewh[tm]
                lidx_all[row:row + K] = flat_l.reshape(K, P)
                dstl_all[row:row + K] = flat_d.reshape(K, P)
                w_all[row:row + K] = flat_w.reshape(K, P)

        # dinv folded into x: g1 = dinv*(x@W1) == (dinv*x)@W1 row-wise
        xs = x[s * NSH + perms[s]] * dinv[s * NSH + perms[s]][:, None]
        xT = np.zeros((P, NPAD), np.float32)
        xT[:, :NSH] = xs.T
        full = np.ones(NPAD, np.float32)
        full[:NSH] = dinv[s * NSH + perms[s]]
        dv = np.ascontiguousarray(full.reshape(NT, P).T)

        in_maps.append({
            "xT": xT.astype(npbf16),
            "idxw": _wrap_idx(lidx_all.reshape(-1)),
            "dstlT": np.ascontiguousarray(dstl_all.T),
            "wT": np.ascontiguousarray(w_all.T),
            "dinv": dv,
        })
    return in_maps, KT


def kernel(x, edge_index, edge_weight, W1, b1, W2, b2, trace=False):
    in_maps, Ks = _prep_inputs(x, edge_index, edge_weight)
    shared = {
        "W1": np.asarray(W1, np.float32).astype(npbf16),
        "W2": np.asarray(W2, np.float32).astype(npbf16),
        "b1b": np.tile(np.asarray(b1, np.float32)[None, :], (P, 1)),
        "b2b": np.tile(np.asarray(b2, np.float32)[None, :], (P, 1)),
        "ident": np.eye(P, dtype=np.float32).astype(npbf16),
    }
    for im in in_maps:
        im.update(shared)

    with_bias = bool(np.any(shared["b1b"]) or np.any(shared["b2b"]))
    nc = _build_program(Ks, with_bias=with_bias)
    res = run_bass_kernel_spmd(nc, in_maps, core_ids=list(range(NCORES)),
                               trace=trace)
    perms = _prep_inputs.last_perms
    out = np.empty((N, OUT), np.float32)
    for s in range(NCORES):
        o = np.asarray(res.results[s]["out"], np.float32)   # [P, NT*OUT]
        o = o.reshape(P, NT, OUT).transpose(1, 0, 2).reshape(NPAD, OUT)
        out[s * NSH + perms[s]] = o[:NSH]
    kernel.last_results = res
    return out
